# revision 34
# baseline (speedup 1.0000x reference)
"""Trainium2 Bass kernel for the CRA relation module.

Math: the reference computes, per sample,
    phi_x = relu((x@W1+b1)*g1+be1), phi_y likewise,  cat_phi = [phi_x; phi_y]
    A = cat_phi cat_phi^T (symmetric!),  R = [A | A^T] = [A | A]
    W = (cat_phi@W3+b3)@W5a + (R@W4+b4)@W5b + b5
    out = x * W[:196] + y * W[196:]
Because A is symmetric and everything after A is linear into a scalar per
token, the relation pipeline collapses to per-sample matvecs:
    u3 = W3@W5a, u4 = W4@W5b, z = u4[:392]+u4[392:], c0 = b3@W5a+b4@W5b+b5
    s  = u3 + phi_x^T z[:196] + phi_y^T z[196:392]       (768-vector)
    out = x*(phi_x@s + c0) + y*(phi_y@s + c0)
All in bf16 on device (rel err ~5e-3 vs the 2e-2 gate). Data-parallel over
the batch: 16 samples per core on 8 cores.

Layouts: the 768x768 matmuls run feature-major (cin on partitions; host
packs [group, 128, 2*6*392] with [x_a|x_b] 392-col blocks, x blocks then y
blocks). The final per-token reweighting runs token-major (tokens on
partitions) so the per-token weight is a per-partition tensor_scalar
operand: the PE matvec's one-row W result is PE-transposed straight into
PSUM columns, and the host supplies a second token-major copy of x,y
([S, 196, 1536] = [x_feat | y_feat]).

Three-phase software pipeline (mains+s-reduction g / matvec+transpose g-1 /
final multiply g-2) keeps the in-order PE, ACT and DVE streams from ever
stalling on the cross-engine tail chain.
"""

import numpy as np
import ml_dtypes
from contextlib import ExitStack

import concourse.bass as bass
import concourse.tile as tile
import concourse.mybir as mybir
from concourse.bass_utils import run_bass_kernel_spmd

F32 = mybir.dt.float32
BF16 = mybir.dt.bfloat16
NPBF = ml_dtypes.bfloat16
ALU = mybir.AluOpType
ACTF = mybir.ActivationFunctionType

B, N, C = 128, 196, 768
NCORES = 8
S = B // NCORES          # 16 samples per core
G = 2                    # samples per weight pass (moving N = 392)
NG = S // G              # 8 groups per core
DT = C // 128            # 6 feature tiles
W2T = 2 * N              # 392
N0 = 128                 # token-major chunk split: 196 = 128 + 68
N1 = N - N0


def build_bass(c0: float) -> bass.Bass:
    nc = bass.Bass()
    xy_d = nc.declare_dram_parameter("xy", [NG, 128, 2 * DT * W2T], BF16,
                                     isOutput=False)
    xt_d = nc.declare_dram_parameter("xt", [S, N, 2 * C], BF16, isOutput=False)
    w1_d = nc.declare_dram_parameter("w1", [C, C], BF16, isOutput=False)
    w2_d = nc.declare_dram_parameter("w2", [C, C], BF16, isOutput=False)
    # packed small constants: [b1 (DT f32) | b2 (DT f32) | u3 (DT f32) |
    #                           zb (W2T bf16 bitcast to W2T/2 f32)]
    cst_d = nc.declare_dram_parameter("cst", [128, 3 * DT + W2T // 2], F32,
                                      isOutput=False)
    out_d = nc.declare_dram_parameter("out", [S, N, C], BF16, isOutput=True)

    with tile.TileContext(nc) as tc, ExitStack() as ctx:
        const = ctx.enter_context(tc.tile_pool(name="const", bufs=1))
        xin = ctx.enter_context(tc.tile_pool(name="xin", bufs=2))
        xtp = ctx.enter_context(tc.tile_pool(name="xtp", bufs=2))
        phip = ctx.enter_context(tc.tile_pool(name="phi", bufs=3))
        sp = ctx.enter_context(tc.tile_pool(name="sp", bufs=3))
        op = ctx.enter_context(tc.tile_pool(name="op", bufs=2))
        ps = ctx.enter_context(tc.tile_pool(name="ps", bufs=2, space="PSUM"))

        # First group's input first: the first psx accumulation needs it plus
        # the w1 k-tiles; everything else can land later.
        YOFF0 = DT * W2T
        xy0 = xin.tile([128, 2 * DT * W2T], BF16, tag="xy", name="xy")
        # small constants land first (one cheap descriptor); weights and the
        # first group's input interleave per k-tile so psx(k) can start the
        # moment its own tile + input chunk arrive.
        cst = const.tile([128, 3 * DT + W2T // 2], F32, tag="cst")
        nc.sync.dma_start(out=cst[:], in_=cst_d[:, :])
        b1t = cst[:, 0:DT]
        b2t = cst[:, DT:2 * DT]
        u3 = cst[:, 2 * DT:3 * DT]
        zb = cst[:, 3 * DT:].bitcast(BF16)
        w1t = const.tile([128, DT * C], BF16, tag="w1")
        w2t = const.tile([128, DT * C], BF16, tag="w2")
        w1_sb = [w1t[:, k * C:(k + 1) * C] for k in range(DT)]
        w2_sb = [w2t[:, k * C:(k + 1) * C] for k in range(DT)]
        for k in range(DT):
            nc.sync.dma_start(out=w1_sb[k], in_=w1_d[k * 128:(k + 1) * 128, :])
            if k % 2 == 0:
                c = k // 2
                nc.sync.dma_start(out=xy0[:, c * 2 * W2T:(c + 1) * 2 * W2T],
                                  in_=xy_d[0, :, c * 2 * W2T:(c + 1) * 2 * W2T])
        for k in range(DT):
            nc.sync.dma_start(out=w2_sb[k], in_=w2_d[k * 128:(k + 1) * 128, :])
        nc.sync.dma_start(out=xy0[:, YOFF0:2 * YOFF0],
                          in_=xy_d[0, :, YOFF0:2 * YOFF0])
        # Absorb the bias-tile DMA deps into ACT program order now, so the
        # relu evictions later only ever wait on the PE semaphore (the ISA
        # Activation descriptor holds a single sync-wait).
        ident = const.tile([1, 1], F32, tag="ident")
        nc.vector.memset(ident[:], 1.0)
        warm1 = const.tile([128, 1], F32, tag="warm1")
        warm2 = const.tile([128, 1], F32, tag="warm2")
        nc.scalar.activation(warm1[:], b1t[:, 0:1], ACTF.Copy)
        nc.scalar.activation(warm2[:], b2t[:, 0:1], ACTF.Copy)

        YOFF = DT * W2T

        def emit_prelude(g, xyg=None):
            if xyg is None:
                xyg = xin.tile([128, 2 * DT * W2T], BF16, tag="xy", name="xy")
                nc.sync.dma_start(out=xyg[:, 0:YOFF], in_=xy_d[g, :, 0:YOFF])
                nc.sync.dma_start(out=xyg[:, YOFF:2 * YOFF],
                                  in_=xy_d[g, :, YOFF:2 * YOFF])
            # phixy[i][d]: [128, 392] = [phi_x | phi_y] of sample (2g+i), tile d
            phixy = [[phip.tile([128, W2T], BF16, tag=f"phi_{i}_{d}",
                                name=f"phi_{i}_{d}") for d in range(DT)]
                     for i in range(G)]
            t_sbs = [sp.tile([128, DT], BF16, tag=f"t_{i}", name=f"t_{i}")
                     for i in range(G)]
            s_sbs = [sp.tile([128, DT], BF16, tag=f"s_{i}", name=f"s_{i}")
                     for i in range(G)]
            return xyg, phixy, t_sbs, s_sbs

        def emit_mains_d(g, st, d_lo, d_hi):
            xyg, phixy, t_sbs, s_sbs = st
            for d in range(d_lo, d_hi):
                psx = ps.tile([128, W2T], F32, tag="psx", name="psx", bufs=2)
                psy = ps.tile([128, W2T], F32, tag="psy", name="psy", bufs=2)
                for k in range(DT):
                    nc.tensor.matmul(
                        psx[:], w1_sb[k][:, d * 128:(d + 1) * 128],
                        xyg[:, k * W2T:(k + 1) * W2T],
                        start=(k == 0), stop=(k == DT - 1))
                for k in range(DT):
                    nc.tensor.matmul(
                        psy[:], w2_sb[k][:, d * 128:(d + 1) * 128],
                        xyg[:, YOFF + k * W2T:YOFF + (k + 1) * W2T],
                        start=(k == 0), stop=(k == DT - 1))
                for i in range(G):
                    nc.scalar.activation(phixy[i][d][:, 0:N], psx[:, i * N:(i + 1) * N],
                                         ACTF.Relu, bias=b1t[:, d:d + 1])
                    nc.scalar.activation(phixy[i][d][:, N:W2T], psy[:, i * N:(i + 1) * N],
                                         ACTF.Relu, bias=b2t[:, d:d + 1])
                # s-reduction rides along per-d: DVE drains while PE moves on
                for i in range(G):
                    scr = sp.tile([128, W2T], BF16, tag=f"scr_{d}", name="scr")
                    nc.vector.scalar_tensor_tensor(
                        out=scr[:], in0=phixy[i][d][:], scalar=1.0, in1=zb[:],
                        op0=ALU.mult, op1=ALU.mult,
                        accum_out=t_sbs[i][:, d:d + 1])

        def emit_mains_fin(g, st):
            xyg, phixy, t_sbs, s_sbs = st
            for i in range(G):
                nc.vector.tensor_tensor(s_sbs[i][:], t_sbs[i][:], u3[:], ALU.add)
            return xyg, s_sbs, phixy

        def emit_tail_a(g, xyg, s_sbs, phixy):
            """PE matvec, W-row transposed into PSUM columns; xt prefetch."""
            pst = ps.tile([128, 4 * G], F32, tag="pst", name="pst", bufs=2)
            xts = []
            for i in range(G):
                sidx = G * g + i
                xt0 = xtp.tile([N0, 2 * C], BF16, tag=f"xt0_{i}", name=f"xt0_{i}")
                xt1 = xtp.tile([N1, 2 * C], BF16, tag=f"xt1_{i}", name=f"xt1_{i}")
                nc.sync.dma_start(out=xt0[:], in_=xt_d[sidx, 0:N0, :])
                nc.sync.dma_start(out=xt1[:], in_=xt_d[sidx, N0:N, :])
                xts.append((xt0, xt1))
                psw = ps.tile([128, W2T], F32, tag="psw", name="psw", bufs=1)
                for d in range(DT):
                    nc.tensor.matmul(psw[0:1, :], s_sbs[i][:, d:d + 1],
                                     phixy[i][d][:],
                                     start=(d == 0), stop=(d == DT - 1))
                wrow = sp.tile([1, W2T], F32, tag="wrow", name="wrow")
                nc.scalar.activation(wrow[:], psw[0:1, :], ACTF.Copy, bias=c0)
                b = 4 * i
                nc.tensor.transpose(pst[:, b + 0:b + 1], wrow[0:1, 0:N0], ident[:])
                nc.tensor.transpose(pst[:, b + 1:b + 2], wrow[0:1, N:N + N0], ident[:])
                nc.tensor.transpose(pst[0:N1, b + 2:b + 3], wrow[0:1, N0:N], ident[:])
                nc.tensor.transpose(pst[0:N1, b + 3:b + 4], wrow[0:1, N + N0:W2T], ident[:])
            return xts, [pst] * G

        def emit_tail_b(g, xts, wcs):
            """out[tok] = x[tok]*(Wx+c0) + y[tok]*(Wy+c0), token-major."""
            for i in range(G):
                (xt0, xt1), pst = xts[i], wcs[i]
                sidx = G * g + i
                b = 4 * i
                for ci, (xt, p0, pn) in enumerate(((xt0, 0, N0), (xt1, N0, N1))):
                    ot = op.tile([pn, C], BF16, tag=f"ot{ci}_{i}",
                                 name=f"ot{ci}_{i}")
                    tmp = op.tile([pn, C], BF16, tag=f"tm{ci}_{i}",
                                  name=f"tm{ci}_{i}")
                    nc.vector.tensor_scalar_mul(
                        tmp[:], xt[:, 0:C], pst[0:pn, b + 2 * ci:b + 2 * ci + 1])
                    nc.vector.scalar_tensor_tensor(
                        out=ot[:], in0=xt[:, C:2 * C],
                        scalar=pst[0:pn, b + 2 * ci + 1:b + 2 * ci + 2],
                        in1=tmp[:], op0=ALU.mult, op1=ALU.add)
                    nc.sync.dma_start(out=out_d[sidx, p0:p0 + pn, :], in_=ot[:])

        # Three-phase pipeline (lags 1/2). tail_a(g-1) is emitted in the
        # middle of mains(g) so its PE matvec/transposes (and the DVE finals
        # they feed) run well before the drain instead of after the last
        # main matmul.
        mains, tails = {}, {}
        for g in range(NG):
            st = emit_prelude(g, xy0 if g == 0 else None)
            emit_mains_d(g, st, 0, DT // 2)
            if g - 1 in mains:
                tails[g - 1] = emit_tail_a(g - 1, *mains.pop(g - 1))
            emit_mains_d(g, st, DT // 2, DT)
            mains[g] = emit_mains_fin(g, st)
            if g - 2 in tails:
                emit_tail_b(g - 2, *tails.pop(g - 2))
        for g in sorted(mains):
            tails[g] = emit_tail_a(g, *mains.pop(g))
        for g in sorted(tails):
            emit_tail_b(g, *tails.pop(g))

    _split_multi_waits(nc)
    return nc


def _split_multi_waits(nc):
    """This walrus build accepts at most ONE sync-wait command per TPB
    instruction; the Tile scheduler happily emits several. Hoist all but the
    last wait of each instruction onto same-engine EventSemaphore ops placed
    immediately before it (engine program order is the within-block
    subsequence, so this preserves semantics)."""
    import json
    data = json.loads(nc.to_json_bytes())
    n = 0
    for fn in data["functions"]:
        for blk in fn["blocks"]:
            out = []
            for inst in blk["instructions"]:
                si = inst.get("sync_info")
                ow = (si or {}).get("on_wait") or []
                if len(ow) > 1:
                    for w in ow[:-1]:
                        n += 1
                        out.append({
                            "name": f"eswait_{n}",
                            "opcode": "EventSemaphore",
                            "engine": inst["engine"],
                            "ins": [],
                            "outs": [],
                            "sync_info": {"on_wait": [w], "on_update": []},
                        })
                    si["on_wait"] = [ow[-1]]
                out.append(inst)
            blk["instructions"] = out
    nc.m = mybir.module_from_json_bytes(json.dumps(data).encode())
    return nc


def prep_host(inputs: dict):
    x = np.ascontiguousarray(np.asarray(inputs["x"], dtype=np.float32))
    y = np.ascontiguousarray(np.asarray(inputs["y"], dtype=np.float32))
    W1 = np.asarray(inputs["W1"], dtype=np.float32)
    W2 = np.asarray(inputs["W2"], dtype=np.float32)
    g1 = np.asarray(inputs["g1"], dtype=np.float32)
    g2 = np.asarray(inputs["g2"], dtype=np.float32)
    b1 = np.asarray(inputs["b1"], dtype=np.float32)
    b2 = np.asarray(inputs["b2"], dtype=np.float32)
    be1 = np.asarray(inputs["be1"], dtype=np.float32)
    be2 = np.asarray(inputs["be2"], dtype=np.float32)
    W3 = np.asarray(inputs["W3"], dtype=np.float32)
    b3 = np.asarray(inputs["b3"], dtype=np.float32)
    W4 = np.asarray(inputs["W4"], dtype=np.float32)
    b4 = np.asarray(inputs["b4"], dtype=np.float32)
    W5 = np.asarray(inputs["W5"], dtype=np.float32)
    b5 = np.asarray(inputs["b5"], dtype=np.float32)

    W1p = np.ascontiguousarray(W1 * g1[None, :]).astype(NPBF)
    W2p = np.ascontiguousarray(W2 * g2[None, :]).astype(NPBF)
    b1p = b1 * g1 + be1
    b2p = b2 * g2 + be2
    W5a, W5b = W5[:C, 0], W5[C:, 0]
    u3 = (W3 @ W5a).astype(np.float32)
    u4 = (W4 @ W5b).astype(np.float32)
    z = (u4[:2 * N] + u4[2 * N:]).astype(np.float32)
    c0 = float(b3 @ W5a + b4 @ W5b + b5[0])

    # [B,N,C] -> per-core groups [M, NG, 128, DT*392] with [x_a|x_b] 392-blocks
    def pack(a):
        at = a.transpose(0, 2, 1).reshape(NCORES, S, DT, 128, N)
        pair = at.reshape(NCORES, NG, G, DT, 128, N)
        gg = np.concatenate([pair[:, :, 0], pair[:, :, 1]], axis=-1)  # [M,NG,DT,128,392]
        return np.ascontiguousarray(
            gg.transpose(0, 1, 3, 2, 4).reshape(NCORES, NG, 128, DT * W2T))

    XY = np.concatenate([pack(x), pack(y)], axis=-1).astype(NPBF)
    # token-major second copy for the final reweighting: [M, S, N, 2C]
    XT = np.concatenate([x, y], axis=-1).reshape(NCORES, S, N, 2 * C).astype(NPBF)
    zb = np.broadcast_to(z[None, :], (128, W2T)).astype(NPBF)
    u3t = u3.reshape(DT, 128).T
    b1t = b1p.reshape(DT, 128).T
    b2t = b2p.reshape(DT, 128).T
    cst = np.concatenate(
        [b1t.astype(np.float32), b2t.astype(np.float32),
         u3t.astype(np.float32),
         np.ascontiguousarray(zb).view(np.float32)], axis=1)
    cst = np.ascontiguousarray(cst)

    in_maps = []
    for cidx in range(NCORES):
        in_maps.append({
            "xy": XY[cidx], "xt": XT[cidx], "w1": W1p, "w2": W2p,
            "cst": cst,
        })
    return in_maps, c0, x, y


def unpack_out(results) -> np.ndarray:
    outs = []
    for cidx in range(NCORES):
        o = np.asarray(results[cidx]["out"]).astype(np.float32)  # [S, N, C]
        outs.append(o)
    return np.ascontiguousarray(np.concatenate(outs, axis=0))


def kernel(**inputs) -> np.ndarray:
    in_maps, c0, _, _ = prep_host(inputs)
    nc = build_bass(c0)
    res = run_bass_kernel_spmd(nc, in_maps, list(range(NCORES)))
    return unpack_out(res.results)


# revision 35
# speedup vs baseline: 1.0015x; 1.0015x over previous
"""Trainium2 Bass kernel for the CRA relation module.

Math: the reference computes, per sample,
    phi_x = relu((x@W1+b1)*g1+be1), phi_y likewise,  cat_phi = [phi_x; phi_y]
    A = cat_phi cat_phi^T (symmetric!),  R = [A | A^T] = [A | A]
    W = (cat_phi@W3+b3)@W5a + (R@W4+b4)@W5b + b5
    out = x * W[:196] + y * W[196:]
Because A is symmetric and everything after A is linear into a scalar per
token, the relation pipeline collapses to per-sample matvecs:
    u3 = W3@W5a, u4 = W4@W5b, z = u4[:392]+u4[392:], c0 = b3@W5a+b4@W5b+b5
    s  = u3 + phi_x^T z[:196] + phi_y^T z[196:392]       (768-vector)
    out = x*(phi_x@s + c0) + y*(phi_y@s + c0)
All in bf16 on device (rel err ~5e-3 vs the 2e-2 gate). Data-parallel over
the batch: 16 samples per core on 8 cores.

Layouts: the 768x768 matmuls run feature-major (cin on partitions; host
packs [group, 128, 2*6*392] with [x_a|x_b] 392-col blocks, x blocks then y
blocks). The final per-token reweighting runs token-major (tokens on
partitions) so the per-token weight is a per-partition tensor_scalar
operand: the PE matvec's one-row W result is PE-transposed straight into
PSUM columns, and the host supplies a second token-major copy of x,y
([S, 196, 1536] = [x_feat | y_feat]).

Three-phase software pipeline (mains+s-reduction g / matvec+transpose g-1 /
final multiply g-2) keeps the in-order PE, ACT and DVE streams from ever
stalling on the cross-engine tail chain.
"""

import numpy as np
import ml_dtypes
from contextlib import ExitStack

import concourse.bass as bass
import concourse.tile as tile
import concourse.mybir as mybir
from concourse.bass_utils import run_bass_kernel_spmd

F32 = mybir.dt.float32
BF16 = mybir.dt.bfloat16
NPBF = ml_dtypes.bfloat16
ALU = mybir.AluOpType
ACTF = mybir.ActivationFunctionType

B, N, C = 128, 196, 768
NCORES = 8
S = B // NCORES          # 16 samples per core
G = 2                    # samples per weight pass (moving N = 392)
NG = S // G              # 8 groups per core
DT = C // 128            # 6 feature tiles
W2T = 2 * N              # 392
N0 = 128                 # token-major chunk split: 196 = 128 + 68
N1 = N - N0


def build_bass(c0: float) -> bass.Bass:
    nc = bass.Bass()
    xy_d = nc.declare_dram_parameter("xy", [NG, 128, 2 * DT * W2T], BF16,
                                     isOutput=False)
    xt_d = nc.declare_dram_parameter("xt", [S, N, 2 * C], BF16, isOutput=False)
    w1_d = nc.declare_dram_parameter("w1", [C, C], BF16, isOutput=False)
    w2_d = nc.declare_dram_parameter("w2", [C, C], BF16, isOutput=False)
    # packed small constants: [b1 (DT f32) | b2 (DT f32) | u3 (DT f32) |
    #                           zb (W2T bf16 bitcast to W2T/2 f32)]
    cst_d = nc.declare_dram_parameter("cst", [128, 3 * DT + W2T // 2], F32,
                                      isOutput=False)
    out_d = nc.declare_dram_parameter("out", [S, N, C], BF16, isOutput=True)

    with tile.TileContext(nc) as tc, ExitStack() as ctx:
        const = ctx.enter_context(tc.tile_pool(name="const", bufs=1))
        xin = ctx.enter_context(tc.tile_pool(name="xin", bufs=2))
        xtp = ctx.enter_context(tc.tile_pool(name="xtp", bufs=2))
        phip = ctx.enter_context(tc.tile_pool(name="phi", bufs=3))
        sp = ctx.enter_context(tc.tile_pool(name="sp", bufs=3))
        op = ctx.enter_context(tc.tile_pool(name="op", bufs=2))
        ps = ctx.enter_context(tc.tile_pool(name="ps", bufs=2, space="PSUM"))

        # First group's input first: the first psx accumulation needs it plus
        # the w1 k-tiles; everything else can land later.
        YOFF0 = DT * W2T
        xy0 = xin.tile([128, 2 * DT * W2T], BF16, tag="xy", name="xy")
        # small constants land first (one cheap descriptor); weights and the
        # first group's input interleave per k-tile so psx(k) can start the
        # moment its own tile + input chunk arrive.
        cst = const.tile([128, 3 * DT + W2T // 2], F32, tag="cst")
        nc.sync.dma_start(out=cst[:], in_=cst_d[:, :])
        b1t = cst[:, 0:DT]
        b2t = cst[:, DT:2 * DT]
        u3 = cst[:, 2 * DT:3 * DT]
        zb = cst[:, 3 * DT:].bitcast(BF16)
        w1t = const.tile([128, DT * C], BF16, tag="w1")
        w2t = const.tile([128, DT * C], BF16, tag="w2")
        w1_sb = [w1t[:, k * C:(k + 1) * C] for k in range(DT)]
        w2_sb = [w2t[:, k * C:(k + 1) * C] for k in range(DT)]
        for k in range(DT):
            nc.sync.dma_start(out=w1_sb[k], in_=w1_d[k * 128:(k + 1) * 128, :])
            if k % 2 == 1:
                c = k // 2
                nc.sync.dma_start(out=xy0[:, c * 2 * W2T:(c + 1) * 2 * W2T],
                                  in_=xy_d[0, :, c * 2 * W2T:(c + 1) * 2 * W2T])
        for k in range(DT):
            nc.sync.dma_start(out=w2_sb[k], in_=w2_d[k * 128:(k + 1) * 128, :])
        nc.sync.dma_start(out=xy0[:, YOFF0:2 * YOFF0],
                          in_=xy_d[0, :, YOFF0:2 * YOFF0])
        # Absorb the bias-tile DMA deps into ACT program order now, so the
        # relu evictions later only ever wait on the PE semaphore (the ISA
        # Activation descriptor holds a single sync-wait).
        ident = const.tile([1, 1], F32, tag="ident")
        nc.vector.memset(ident[:], 1.0)
        warm1 = const.tile([128, 1], F32, tag="warm1")
        warm2 = const.tile([128, 1], F32, tag="warm2")
        nc.scalar.activation(warm1[:], b1t[:, 0:1], ACTF.Copy)
        nc.scalar.activation(warm2[:], b2t[:, 0:1], ACTF.Copy)

        YOFF = DT * W2T

        def emit_prelude(g, xyg=None):
            if xyg is None:
                xyg = xin.tile([128, 2 * DT * W2T], BF16, tag="xy", name="xy")
                nc.sync.dma_start(out=xyg[:, 0:YOFF], in_=xy_d[g, :, 0:YOFF])
                nc.sync.dma_start(out=xyg[:, YOFF:2 * YOFF],
                                  in_=xy_d[g, :, YOFF:2 * YOFF])
            # phixy[i][d]: [128, 392] = [phi_x | phi_y] of sample (2g+i), tile d
            phixy = [[phip.tile([128, W2T], BF16, tag=f"phi_{i}_{d}",
                                name=f"phi_{i}_{d}") for d in range(DT)]
                     for i in range(G)]
            t_sbs = [sp.tile([128, DT], BF16, tag=f"t_{i}", name=f"t_{i}")
                     for i in range(G)]
            s_sbs = [sp.tile([128, DT], BF16, tag=f"s_{i}", name=f"s_{i}")
                     for i in range(G)]
            return xyg, phixy, t_sbs, s_sbs

        def emit_mains_d(g, st, d_lo, d_hi):
            xyg, phixy, t_sbs, s_sbs = st
            last = g == NG - 1
            for d in range(d_lo, d_hi):
                psx = ps.tile([128, W2T], F32, tag="psx", name="psx", bufs=2)
                psy = ps.tile([128, W2T], F32, tag="psy", name="psy", bufs=2)
                for k in range(DT):
                    nc.tensor.matmul(
                        psx[:], w1_sb[k][:, d * 128:(d + 1) * 128],
                        xyg[:, k * W2T:(k + 1) * W2T],
                        start=(k == 0), stop=(k == DT - 1))
                for k in range(DT):
                    nc.tensor.matmul(
                        psy[:], w2_sb[k][:, d * 128:(d + 1) * 128],
                        xyg[:, YOFF + k * W2T:YOFF + (k + 1) * W2T],
                        start=(k == 0), stop=(k == DT - 1))
                for i in range(G):
                    nc.scalar.activation(phixy[i][d][:, 0:N], psx[:, i * N:(i + 1) * N],
                                         ACTF.Relu, bias=b1t[:, d:d + 1])
                    nc.scalar.activation(phixy[i][d][:, N:W2T], psy[:, i * N:(i + 1) * N],
                                         ACTF.Relu, bias=b2t[:, d:d + 1])
                # s-reduction rides along per-d: DVE drains while PE moves on
                for i in range(G):
                    scr = sp.tile([128, W2T], BF16, tag=f"scr_{d}", name="scr")
                    nc.vector.scalar_tensor_tensor(
                        out=scr[:], in0=phixy[i][d][:], scalar=1.0, in1=zb[:],
                        op0=ALU.mult, op1=ALU.mult,
                        accum_out=t_sbs[i][:, d:d + 1])
                    if last:
                        # final group: publish s_d immediately so the drain's
                        # matvec never waits on the batched s add
                        nc.vector.tensor_scalar_add(
                            s_sbs[i][:, d:d + 1], t_sbs[i][:, d:d + 1],
                            u3[:, d:d + 1])

        def emit_mains_fin(g, st):
            xyg, phixy, t_sbs, s_sbs = st
            if g != NG - 1:
                for i in range(G):
                    nc.vector.tensor_tensor(s_sbs[i][:], t_sbs[i][:], u3[:],
                                            ALU.add)
            return xyg, s_sbs, phixy

        def emit_tail_a(g, xyg, s_sbs, phixy):
            """PE matvec, W-row transposed into PSUM columns; xt prefetch."""
            pst = ps.tile([128, 4 * G], F32, tag="pst", name="pst", bufs=2)
            xts = []
            for i in range(G):
                sidx = G * g + i
                xt0 = xtp.tile([N0, 2 * C], BF16, tag=f"xt0_{i}", name=f"xt0_{i}")
                xt1 = xtp.tile([N1, 2 * C], BF16, tag=f"xt1_{i}", name=f"xt1_{i}")
                nc.sync.dma_start(out=xt0[:], in_=xt_d[sidx, 0:N0, :])
                nc.sync.dma_start(out=xt1[:], in_=xt_d[sidx, N0:N, :])
                xts.append((xt0, xt1))
                psw = ps.tile([128, W2T], F32, tag="psw", name="psw", bufs=1)
                for d in range(DT):
                    nc.tensor.matmul(psw[0:1, :], s_sbs[i][:, d:d + 1],
                                     phixy[i][d][:],
                                     start=(d == 0), stop=(d == DT - 1))
                wrow = sp.tile([1, W2T], F32, tag="wrow", name="wrow")
                nc.scalar.activation(wrow[:], psw[0:1, :], ACTF.Copy, bias=c0)
                b = 4 * i
                nc.tensor.transpose(pst[:, b + 0:b + 1], wrow[0:1, 0:N0], ident[:])
                nc.tensor.transpose(pst[:, b + 1:b + 2], wrow[0:1, N:N + N0], ident[:])
                nc.tensor.transpose(pst[0:N1, b + 2:b + 3], wrow[0:1, N0:N], ident[:])
                nc.tensor.transpose(pst[0:N1, b + 3:b + 4], wrow[0:1, N + N0:W2T], ident[:])
            return xts, [pst] * G

        def emit_tail_b(g, xts, wcs):
            """out[tok] = x[tok]*(Wx+c0) + y[tok]*(Wy+c0), token-major."""
            for i in range(G):
                (xt0, xt1), pst = xts[i], wcs[i]
                sidx = G * g + i
                b = 4 * i
                for ci, (xt, p0, pn) in enumerate(((xt0, 0, N0), (xt1, N0, N1))):
                    ot = op.tile([pn, C], BF16, tag=f"ot{ci}_{i}",
                                 name=f"ot{ci}_{i}")
                    tmp = op.tile([pn, C], BF16, tag=f"tm{ci}_{i}",
                                  name=f"tm{ci}_{i}")
                    nc.vector.tensor_scalar_mul(
                        tmp[:], xt[:, 0:C], pst[0:pn, b + 2 * ci:b + 2 * ci + 1])
                    nc.vector.scalar_tensor_tensor(
                        out=ot[:], in0=xt[:, C:2 * C],
                        scalar=pst[0:pn, b + 2 * ci + 1:b + 2 * ci + 2],
                        in1=tmp[:], op0=ALU.mult, op1=ALU.add)
                    nc.sync.dma_start(out=out_d[sidx, p0:p0 + pn, :], in_=ot[:])

        # Three-phase pipeline (lags 1/2). tail_a(g-1) is emitted in the
        # middle of mains(g) so its PE matvec/transposes (and the DVE finals
        # they feed) run well before the drain instead of after the last
        # main matmul.
        mains, tails = {}, {}
        for g in range(NG):
            st = emit_prelude(g, xy0 if g == 0 else None)
            emit_mains_d(g, st, 0, DT // 2)
            if g - 1 in mains:
                tails[g - 1] = emit_tail_a(g - 1, *mains.pop(g - 1))
            emit_mains_d(g, st, DT // 2, DT)
            mains[g] = emit_mains_fin(g, st)
            if g - 2 in tails:
                emit_tail_b(g - 2, *tails.pop(g - 2))
        for g in sorted(mains):
            tails[g] = emit_tail_a(g, *mains.pop(g))
        for g in sorted(tails):
            emit_tail_b(g, *tails.pop(g))

    _split_multi_waits(nc)
    return nc


def _split_multi_waits(nc):
    """This walrus build accepts at most ONE sync-wait command per TPB
    instruction; the Tile scheduler happily emits several. Hoist all but the
    last wait of each instruction onto same-engine EventSemaphore ops placed
    immediately before it (engine program order is the within-block
    subsequence, so this preserves semantics)."""
    import json
    data = json.loads(nc.to_json_bytes())
    n = 0
    for fn in data["functions"]:
        for blk in fn["blocks"]:
            out = []
            for inst in blk["instructions"]:
                si = inst.get("sync_info")
                ow = (si or {}).get("on_wait") or []
                if len(ow) > 1:
                    for w in ow[:-1]:
                        n += 1
                        out.append({
                            "name": f"eswait_{n}",
                            "opcode": "EventSemaphore",
                            "engine": inst["engine"],
                            "ins": [],
                            "outs": [],
                            "sync_info": {"on_wait": [w], "on_update": []},
                        })
                    si["on_wait"] = [ow[-1]]
                out.append(inst)
            blk["instructions"] = out
    nc.m = mybir.module_from_json_bytes(json.dumps(data).encode())
    return nc


def prep_host(inputs: dict):
    x = np.ascontiguousarray(np.asarray(inputs["x"], dtype=np.float32))
    y = np.ascontiguousarray(np.asarray(inputs["y"], dtype=np.float32))
    W1 = np.asarray(inputs["W1"], dtype=np.float32)
    W2 = np.asarray(inputs["W2"], dtype=np.float32)
    g1 = np.asarray(inputs["g1"], dtype=np.float32)
    g2 = np.asarray(inputs["g2"], dtype=np.float32)
    b1 = np.asarray(inputs["b1"], dtype=np.float32)
    b2 = np.asarray(inputs["b2"], dtype=np.float32)
    be1 = np.asarray(inputs["be1"], dtype=np.float32)
    be2 = np.asarray(inputs["be2"], dtype=np.float32)
    W3 = np.asarray(inputs["W3"], dtype=np.float32)
    b3 = np.asarray(inputs["b3"], dtype=np.float32)
    W4 = np.asarray(inputs["W4"], dtype=np.float32)
    b4 = np.asarray(inputs["b4"], dtype=np.float32)
    W5 = np.asarray(inputs["W5"], dtype=np.float32)
    b5 = np.asarray(inputs["b5"], dtype=np.float32)

    W1p = np.ascontiguousarray(W1 * g1[None, :]).astype(NPBF)
    W2p = np.ascontiguousarray(W2 * g2[None, :]).astype(NPBF)
    b1p = b1 * g1 + be1
    b2p = b2 * g2 + be2
    W5a, W5b = W5[:C, 0], W5[C:, 0]
    u3 = (W3 @ W5a).astype(np.float32)
    u4 = (W4 @ W5b).astype(np.float32)
    z = (u4[:2 * N] + u4[2 * N:]).astype(np.float32)
    c0 = float(b3 @ W5a + b4 @ W5b + b5[0])

    # [B,N,C] -> per-core groups [M, NG, 128, DT*392] with [x_a|x_b] 392-blocks
    def pack(a):
        at = a.transpose(0, 2, 1).reshape(NCORES, S, DT, 128, N)
        pair = at.reshape(NCORES, NG, G, DT, 128, N)
        gg = np.concatenate([pair[:, :, 0], pair[:, :, 1]], axis=-1)  # [M,NG,DT,128,392]
        return np.ascontiguousarray(
            gg.transpose(0, 1, 3, 2, 4).reshape(NCORES, NG, 128, DT * W2T))

    XY = np.concatenate([pack(x), pack(y)], axis=-1).astype(NPBF)
    # token-major second copy for the final reweighting: [M, S, N, 2C]
    XT = np.concatenate([x, y], axis=-1).reshape(NCORES, S, N, 2 * C).astype(NPBF)
    zb = np.broadcast_to(z[None, :], (128, W2T)).astype(NPBF)
    u3t = u3.reshape(DT, 128).T
    b1t = b1p.reshape(DT, 128).T
    b2t = b2p.reshape(DT, 128).T
    cst = np.concatenate(
        [b1t.astype(np.float32), b2t.astype(np.float32),
         u3t.astype(np.float32),
         np.ascontiguousarray(zb).view(np.float32)], axis=1)
    cst = np.ascontiguousarray(cst)

    in_maps = []
    for cidx in range(NCORES):
        in_maps.append({
            "xy": XY[cidx], "xt": XT[cidx], "w1": W1p, "w2": W2p,
            "cst": cst,
        })
    return in_maps, c0, x, y


def unpack_out(results) -> np.ndarray:
    outs = []
    for cidx in range(NCORES):
        o = np.asarray(results[cidx]["out"]).astype(np.float32)  # [S, N, C]
        outs.append(o)
    return np.ascontiguousarray(np.concatenate(outs, axis=0))


def kernel(**inputs) -> np.ndarray:
    in_maps, c0, _, _ = prep_host(inputs)
    nc = build_bass(c0)
    res = run_bass_kernel_spmd(nc, in_maps, list(range(NCORES)))
    return unpack_out(res.results)


# revision 36
# speedup vs baseline: 1.0191x; 1.0176x over previous
"""Trainium2 Bass kernel for the CRA relation module.

Math: the reference computes, per sample,
    phi_x = relu((x@W1+b1)*g1+be1), phi_y likewise,  cat_phi = [phi_x; phi_y]
    A = cat_phi cat_phi^T (symmetric!),  R = [A | A^T] = [A | A]
    W = (cat_phi@W3+b3)@W5a + (R@W4+b4)@W5b + b5
    out = x * W[:196] + y * W[196:]
Because A is symmetric and everything after A is linear into a scalar per
token, the relation pipeline collapses to per-sample matvecs:
    u3 = W3@W5a, u4 = W4@W5b, z = u4[:392]+u4[392:], c0 = b3@W5a+b4@W5b+b5
    s  = u3 + phi_x^T z[:196] + phi_y^T z[196:392]       (768-vector)
    out = x*(phi_x@s + c0) + y*(phi_y@s + c0)
All in bf16 on device (rel err ~5e-3 vs the 2e-2 gate). Data-parallel over
the batch: 16 samples per core on 8 cores.

Layouts: the 768x768 matmuls run feature-major (cin on partitions; host
packs [group, 128, 2*6*392] with [x_a|x_b] 392-col blocks, x blocks then y
blocks). The final per-token reweighting runs token-major (tokens on
partitions) so the per-token weight is a per-partition tensor_scalar
operand: the PE matvec's one-row W result is PE-transposed straight into
PSUM columns, and the host supplies a second token-major copy of x,y
([S, 196, 1536] = [x_feat | y_feat]).

Three-phase software pipeline (mains+s-reduction g / matvec+transpose g-1 /
final multiply g-2) keeps the in-order PE, ACT and DVE streams from ever
stalling on the cross-engine tail chain.
"""

import numpy as np
import ml_dtypes
from contextlib import ExitStack

import concourse.bass as bass
import concourse.tile as tile
import concourse.mybir as mybir
from concourse.bass_utils import run_bass_kernel_spmd

F32 = mybir.dt.float32
BF16 = mybir.dt.bfloat16
NPBF = ml_dtypes.bfloat16
ALU = mybir.AluOpType
ACTF = mybir.ActivationFunctionType

B, N, C = 128, 196, 768
NCORES = 8
S = B // NCORES          # 16 samples per core
G = 2                    # samples per weight pass (moving N = 392)
NG = S // G              # 8 groups per core
DT = C // 128            # 6 feature tiles
W2T = 2 * N              # 392
N0 = 128                 # token-major chunk split: 196 = 128 + 68
N1 = N - N0


def build_bass(c0: float) -> bass.Bass:
    nc = bass.Bass()
    xy_d = nc.declare_dram_parameter("xy", [NG, 128, 2 * DT * W2T], BF16,
                                     isOutput=False)
    xt_d = nc.declare_dram_parameter("xt", [S, N, 2 * C], BF16, isOutput=False)
    w1_d = nc.declare_dram_parameter("w1", [C, C], BF16, isOutput=False)
    w2_d = nc.declare_dram_parameter("w2", [C, C], BF16, isOutput=False)
    # packed small constants: [b1 (DT f32) | b2 (DT f32) | u3 (DT f32) |
    #                           zb (W2T bf16 bitcast to W2T/2 f32)]
    cst_d = nc.declare_dram_parameter("cst", [128, 3 * DT + W2T // 2], F32,
                                      isOutput=False)
    out_d = nc.declare_dram_parameter("out", [S, N, C], BF16, isOutput=True)

    with tile.TileContext(nc) as tc, ExitStack() as ctx:
        const = ctx.enter_context(tc.tile_pool(name="const", bufs=1))
        xin = ctx.enter_context(tc.tile_pool(name="xin", bufs=2))
        xtp = ctx.enter_context(tc.tile_pool(name="xtp", bufs=2))
        phip = ctx.enter_context(tc.tile_pool(name="phi", bufs=3))
        sp = ctx.enter_context(tc.tile_pool(name="sp", bufs=3))
        op = ctx.enter_context(tc.tile_pool(name="op", bufs=2))
        ps = ctx.enter_context(tc.tile_pool(name="ps", bufs=2, space="PSUM"))

        # First group's input first: the first psx accumulation needs it plus
        # the w1 k-tiles; everything else can land later.
        YOFF0 = DT * W2T
        xy0 = xin.tile([128, 2 * DT * W2T], BF16, tag="xy", name="xy")
        # small constants land first (one cheap descriptor); weights and the
        # first group's input interleave per k-tile so psx(k) can start the
        # moment its own tile + input chunk arrive.
        cst = const.tile([128, 3 * DT + W2T // 2], F32, tag="cst")
        nc.sync.dma_start(out=cst[:], in_=cst_d[:, :])
        b1t = cst[:, 0:DT]
        b2t = cst[:, DT:2 * DT]
        u3 = cst[:, 2 * DT:3 * DT]
        zb = cst[:, 3 * DT:].bitcast(BF16)
        w1t = const.tile([128, DT * C], BF16, tag="w1")
        w2t = const.tile([128, DT * C], BF16, tag="w2")
        w1_sb = [w1t[:, k * C:(k + 1) * C] for k in range(DT)]
        w2_sb = [w2t[:, k * C:(k + 1) * C] for k in range(DT)]
        for k in range(DT):
            nc.sync.dma_start(out=w1_sb[k], in_=w1_d[k * 128:(k + 1) * 128, :])
            if k % 2 == 0:
                c = k // 2
                nc.sync.dma_start(out=xy0[:, c * 2 * W2T:(c + 1) * 2 * W2T],
                                  in_=xy_d[0, :, c * 2 * W2T:(c + 1) * 2 * W2T])
        for k in range(DT):
            nc.sync.dma_start(out=w2_sb[k], in_=w2_d[k * 128:(k + 1) * 128, :])
        nc.sync.dma_start(out=xy0[:, YOFF0:2 * YOFF0],
                          in_=xy_d[0, :, YOFF0:2 * YOFF0])
        # Absorb the bias-tile DMA deps into ACT program order now, so the
        # relu evictions later only ever wait on the PE semaphore (the ISA
        # Activation descriptor holds a single sync-wait).
        ident = const.tile([1, 1], F32, tag="ident")
        nc.vector.memset(ident[:], 1.0)
        warm1 = const.tile([128, 1], F32, tag="warm1")
        warm2 = const.tile([128, 1], F32, tag="warm2")
        nc.scalar.activation(warm1[:], b1t[:, 0:1], ACTF.Copy)
        nc.scalar.activation(warm2[:], b2t[:, 0:1], ACTF.Copy)

        YOFF = DT * W2T

        def emit_prelude(g, xyg=None):
            if xyg is None:
                xyg = xin.tile([128, 2 * DT * W2T], BF16, tag="xy", name="xy")
                nc.sync.dma_start(out=xyg[:, 0:YOFF], in_=xy_d[g, :, 0:YOFF])
                nc.sync.dma_start(out=xyg[:, YOFF:2 * YOFF],
                                  in_=xy_d[g, :, YOFF:2 * YOFF])
            # phixy[i][d]: [128, 392] = [phi_x | phi_y] of sample (2g+i), tile d
            phixy = [[phip.tile([128, W2T], BF16, tag=f"phi_{i}_{d}",
                                name=f"phi_{i}_{d}") for d in range(DT)]
                     for i in range(G)]
            t_sbs = [sp.tile([128, DT], F32, tag=f"t_{i}", name=f"t_{i}")
                     for i in range(G)]
            s_sbs = [sp.tile([128, DT], BF16, tag=f"s_{i}", name=f"s_{i}")
                     for i in range(G)]
            return xyg, phixy, t_sbs, s_sbs

        def emit_mains_d(g, st, d_lo, d_hi):
            xyg, phixy, t_sbs, s_sbs = st
            for d in range(d_lo, d_hi):
                psx = ps.tile([128, W2T], F32, tag="psx", name="psx", bufs=2)
                psy = ps.tile([128, W2T], F32, tag="psy", name="psy", bufs=2)
                for k in range(DT):
                    nc.tensor.matmul(
                        psx[:], w1_sb[k][:, d * 128:(d + 1) * 128],
                        xyg[:, k * W2T:(k + 1) * W2T],
                        start=(k == 0), stop=(k == DT - 1))
                for k in range(DT):
                    nc.tensor.matmul(
                        psy[:], w2_sb[k][:, d * 128:(d + 1) * 128],
                        xyg[:, YOFF + k * W2T:YOFF + (k + 1) * W2T],
                        start=(k == 0), stop=(k == DT - 1))
                for i in range(G):
                    nc.scalar.activation(phixy[i][d][:, 0:N], psx[:, i * N:(i + 1) * N],
                                         ACTF.Relu, bias=b1t[:, d:d + 1])
                    nc.scalar.activation(phixy[i][d][:, N:W2T], psy[:, i * N:(i + 1) * N],
                                         ACTF.Relu, bias=b2t[:, d:d + 1])
                # s-reduction rides along per-d: DVE drains while PE moves on
                for i in range(G):
                    scr = sp.tile([128, W2T], BF16, tag=f"scr_{d}", name="scr")
                    nc.vector.scalar_tensor_tensor(
                        out=scr[:], in0=phixy[i][d][:], scalar=1.0, in1=zb[:],
                        op0=ALU.mult, op1=ALU.mult,
                        accum_out=t_sbs[i][:, d:d + 1])

        def emit_mains_fin(g, st):
            xyg, phixy, t_sbs, s_sbs = st
            for i in range(G):
                nc.vector.tensor_tensor(s_sbs[i][:], t_sbs[i][:], u3[:],
                                        ALU.add)
            return xyg, s_sbs, phixy

        def emit_tail_a(g, xyg, s_sbs, phixy):
            """PE matvec, W-row transposed into PSUM columns; xt prefetch."""
            pst = ps.tile([128, 4 * G], F32, tag="pst", name="pst", bufs=2)
            xts = []
            for i in range(G):
                sidx = G * g + i
                xt0 = xtp.tile([N0, 2 * C], BF16, tag=f"xt0_{i}", name=f"xt0_{i}")
                xt1 = xtp.tile([N1, 2 * C], BF16, tag=f"xt1_{i}", name=f"xt1_{i}")
                nc.sync.dma_start(out=xt0[:], in_=xt_d[sidx, 0:N0, :])
                nc.sync.dma_start(out=xt1[:], in_=xt_d[sidx, N0:N, :])
                xts.append((xt0, xt1))
                psw = ps.tile([128, W2T], F32, tag="psw", name="psw", bufs=1)
                for d in range(DT):
                    nc.tensor.matmul(psw[0:1, :], s_sbs[i][:, d:d + 1],
                                     phixy[i][d][:],
                                     start=(d == 0), stop=(d == DT - 1))
                wrow = sp.tile([1, W2T], F32, tag="wrow", name="wrow")
                nc.scalar.activation(wrow[:], psw[0:1, :], ACTF.Copy, bias=c0)
                b = 4 * i
                nc.tensor.transpose(pst[:, b + 0:b + 1], wrow[0:1, 0:N0], ident[:])
                nc.tensor.transpose(pst[:, b + 1:b + 2], wrow[0:1, N:N + N0], ident[:])
                nc.tensor.transpose(pst[0:N1, b + 2:b + 3], wrow[0:1, N0:N], ident[:])
                nc.tensor.transpose(pst[0:N1, b + 3:b + 4], wrow[0:1, N + N0:W2T], ident[:])
            return xts, [pst] * G

        def emit_tail_b(g, xts, wcs):
            """out[tok] = x[tok]*(Wx+c0) + y[tok]*(Wy+c0), token-major."""
            for i in range(G):
                (xt0, xt1), pst = xts[i], wcs[i]
                sidx = G * g + i
                b = 4 * i
                for ci, (xt, p0, pn) in enumerate(((xt0, 0, N0), (xt1, N0, N1))):
                    ot = op.tile([pn, C], BF16, tag=f"ot{ci}_{i}",
                                 name=f"ot{ci}_{i}")
                    tmp = op.tile([pn, C], BF16, tag=f"tm{ci}_{i}",
                                  name=f"tm{ci}_{i}")
                    nc.vector.tensor_scalar_mul(
                        tmp[:], xt[:, 0:C], pst[0:pn, b + 2 * ci:b + 2 * ci + 1])
                    nc.vector.scalar_tensor_tensor(
                        out=ot[:], in0=xt[:, C:2 * C],
                        scalar=pst[0:pn, b + 2 * ci + 1:b + 2 * ci + 2],
                        in1=tmp[:], op0=ALU.mult, op1=ALU.add)
                    nc.sync.dma_start(out=out_d[sidx, p0:p0 + pn, :], in_=ot[:])

        # Three-phase pipeline (lags 1/2): the s-vector is already computed
        # inside mains, so the PE-side tail can follow one group behind.
        mains, tails = {}, {}
        for g in range(NG):
            st = emit_prelude(g, xy0 if g == 0 else None)
            emit_mains_d(g, st, 0, DT)
            mains[g] = emit_mains_fin(g, st)
            if g - 1 in mains:
                tails[g - 1] = emit_tail_a(g - 1, *mains.pop(g - 1))
            if g - 2 in tails:
                emit_tail_b(g - 2, *tails.pop(g - 2))
        for g in sorted(mains):
            tails[g] = emit_tail_a(g, *mains.pop(g))
        for g in sorted(tails):
            emit_tail_b(g, *tails.pop(g))

    _split_multi_waits(nc)
    return nc


def _split_multi_waits(nc):
    """This walrus build accepts at most ONE sync-wait command per TPB
    instruction; the Tile scheduler happily emits several. Hoist all but the
    last wait of each instruction onto same-engine EventSemaphore ops placed
    immediately before it (engine program order is the within-block
    subsequence, so this preserves semantics)."""
    import json
    data = json.loads(nc.to_json_bytes())
    n = 0
    for fn in data["functions"]:
        for blk in fn["blocks"]:
            out = []
            for inst in blk["instructions"]:
                si = inst.get("sync_info")
                ow = (si or {}).get("on_wait") or []
                if len(ow) > 1:
                    for w in ow[:-1]:
                        n += 1
                        out.append({
                            "name": f"eswait_{n}",
                            "opcode": "EventSemaphore",
                            "engine": inst["engine"],
                            "ins": [],
                            "outs": [],
                            "sync_info": {"on_wait": [w], "on_update": []},
                        })
                    si["on_wait"] = [ow[-1]]
                out.append(inst)
            blk["instructions"] = out
    nc.m = mybir.module_from_json_bytes(json.dumps(data).encode())
    return nc


def prep_host(inputs: dict):
    x = np.ascontiguousarray(np.asarray(inputs["x"], dtype=np.float32))
    y = np.ascontiguousarray(np.asarray(inputs["y"], dtype=np.float32))
    W1 = np.asarray(inputs["W1"], dtype=np.float32)
    W2 = np.asarray(inputs["W2"], dtype=np.float32)
    g1 = np.asarray(inputs["g1"], dtype=np.float32)
    g2 = np.asarray(inputs["g2"], dtype=np.float32)
    b1 = np.asarray(inputs["b1"], dtype=np.float32)
    b2 = np.asarray(inputs["b2"], dtype=np.float32)
    be1 = np.asarray(inputs["be1"], dtype=np.float32)
    be2 = np.asarray(inputs["be2"], dtype=np.float32)
    W3 = np.asarray(inputs["W3"], dtype=np.float32)
    b3 = np.asarray(inputs["b3"], dtype=np.float32)
    W4 = np.asarray(inputs["W4"], dtype=np.float32)
    b4 = np.asarray(inputs["b4"], dtype=np.float32)
    W5 = np.asarray(inputs["W5"], dtype=np.float32)
    b5 = np.asarray(inputs["b5"], dtype=np.float32)

    W1p = np.ascontiguousarray(W1 * g1[None, :]).astype(NPBF)
    W2p = np.ascontiguousarray(W2 * g2[None, :]).astype(NPBF)
    b1p = b1 * g1 + be1
    b2p = b2 * g2 + be2
    W5a, W5b = W5[:C, 0], W5[C:, 0]
    u3 = (W3 @ W5a).astype(np.float32)
    u4 = (W4 @ W5b).astype(np.float32)
    z = (u4[:2 * N] + u4[2 * N:]).astype(np.float32)
    c0 = float(b3 @ W5a + b4 @ W5b + b5[0])

    # [B,N,C] -> per-core groups [M, NG, 128, DT*392] with [x_a|x_b] 392-blocks
    def pack(a):
        at = a.transpose(0, 2, 1).reshape(NCORES, S, DT, 128, N)
        pair = at.reshape(NCORES, NG, G, DT, 128, N)
        gg = np.concatenate([pair[:, :, 0], pair[:, :, 1]], axis=-1)  # [M,NG,DT,128,392]
        return np.ascontiguousarray(
            gg.transpose(0, 1, 3, 2, 4).reshape(NCORES, NG, 128, DT * W2T))

    XY = np.concatenate([pack(x), pack(y)], axis=-1).astype(NPBF)
    # token-major second copy for the final reweighting: [M, S, N, 2C]
    XT = np.concatenate([x, y], axis=-1).reshape(NCORES, S, N, 2 * C).astype(NPBF)
    zb = np.broadcast_to(z[None, :], (128, W2T)).astype(NPBF)
    u3t = u3.reshape(DT, 128).T
    b1t = b1p.reshape(DT, 128).T
    b2t = b2p.reshape(DT, 128).T
    cst = np.concatenate(
        [b1t.astype(np.float32), b2t.astype(np.float32),
         u3t.astype(np.float32),
         np.ascontiguousarray(zb).view(np.float32)], axis=1)
    cst = np.ascontiguousarray(cst)

    in_maps = []
    for cidx in range(NCORES):
        in_maps.append({
            "xy": XY[cidx], "xt": XT[cidx], "w1": W1p, "w2": W2p,
            "cst": cst,
        })
    return in_maps, c0, x, y


def unpack_out(results) -> np.ndarray:
    outs = []
    for cidx in range(NCORES):
        o = np.asarray(results[cidx]["out"]).astype(np.float32)  # [S, N, C]
        outs.append(o)
    return np.ascontiguousarray(np.concatenate(outs, axis=0))


def kernel(**inputs) -> np.ndarray:
    in_maps, c0, _, _ = prep_host(inputs)
    nc = build_bass(c0)
    res = run_bass_kernel_spmd(nc, in_maps, list(range(NCORES)))
    return unpack_out(res.results)


# revision 37
# speedup vs baseline: 1.0283x; 1.0090x over previous
"""Trainium2 Bass kernel for the CRA relation module.

Math: the reference computes, per sample,
    phi_x = relu((x@W1+b1)*g1+be1), phi_y likewise,  cat_phi = [phi_x; phi_y]
    A = cat_phi cat_phi^T (symmetric!),  R = [A | A^T] = [A | A]
    W = (cat_phi@W3+b3)@W5a + (R@W4+b4)@W5b + b5
    out = x * W[:196] + y * W[196:]
Because A is symmetric and everything after A is linear into a scalar per
token, the relation pipeline collapses to per-sample matvecs:
    u3 = W3@W5a, u4 = W4@W5b, z = u4[:392]+u4[392:], c0 = b3@W5a+b4@W5b+b5
    s  = u3 + phi_x^T z[:196] + phi_y^T z[196:392]       (768-vector)
    out = x*(phi_x@s + c0) + y*(phi_y@s + c0)
All in bf16 on device (rel err ~5e-3 vs the 2e-2 gate). Data-parallel over
the batch: 16 samples per core on 8 cores.

Layouts: the 768x768 matmuls run feature-major (cin on partitions; host
packs [group, 128, 2*6*392] with [x_a|x_b] 392-col blocks, x blocks then y
blocks). The final per-token reweighting runs token-major (tokens on
partitions) so the per-token weight is a per-partition tensor_scalar
operand: the PE matvec's one-row W result is PE-transposed straight into
PSUM columns, and the host supplies a second token-major copy of x,y
([S, 196, 1536] = [x_feat | y_feat]).

Three-phase software pipeline (mains+s-reduction g / matvec+transpose g-1 /
final multiply g-2) keeps the in-order PE, ACT and DVE streams from ever
stalling on the cross-engine tail chain.
"""

import numpy as np
import ml_dtypes
from contextlib import ExitStack

import concourse.bass as bass
import concourse.tile as tile
import concourse.mybir as mybir
from concourse.bass_utils import run_bass_kernel_spmd

F32 = mybir.dt.float32
BF16 = mybir.dt.bfloat16
NPBF = ml_dtypes.bfloat16
ALU = mybir.AluOpType
ACTF = mybir.ActivationFunctionType

B, N, C = 128, 196, 768
NCORES = 8
S = B // NCORES          # 16 samples per core
G = 2                    # samples per weight pass (moving N = 392)
NG = S // G              # 8 groups per core
DT = C // 128            # 6 feature tiles
W2T = 2 * N              # 392
N0 = 128                 # token-major chunk split: 196 = 128 + 68
N1 = N - N0


def build_bass(c0: float) -> bass.Bass:
    nc = bass.Bass()
    xy_d = nc.declare_dram_parameter("xy", [NG, 128, 2 * DT * W2T], BF16,
                                     isOutput=False)
    xt_d = nc.declare_dram_parameter("xt", [S, N, 2 * C], BF16, isOutput=False)
    w1_d = nc.declare_dram_parameter("w1", [C, C], BF16, isOutput=False)
    w2_d = nc.declare_dram_parameter("w2", [C, C], BF16, isOutput=False)
    # packed small constants: [b1 (DT f32) | b2 (DT f32) | u3 (DT f32) |
    #                           zb (W2T bf16 bitcast to W2T/2 f32)]
    cst_d = nc.declare_dram_parameter("cst", [128, 3 * DT + W2T // 2], F32,
                                      isOutput=False)
    out_d = nc.declare_dram_parameter("out", [S, N, C], BF16, isOutput=True)

    with tile.TileContext(nc) as tc, ExitStack() as ctx:
        const = ctx.enter_context(tc.tile_pool(name="const", bufs=1))
        xin = ctx.enter_context(tc.tile_pool(name="xin", bufs=2))
        xtp = ctx.enter_context(tc.tile_pool(name="xtp", bufs=2))
        phip = ctx.enter_context(tc.tile_pool(name="phi", bufs=3))
        sp = ctx.enter_context(tc.tile_pool(name="sp", bufs=3))
        op = ctx.enter_context(tc.tile_pool(name="op", bufs=2))
        ps = ctx.enter_context(tc.tile_pool(name="ps", bufs=2, space="PSUM"))

        # First group's input first: the first psx accumulation needs it plus
        # the w1 k-tiles; everything else can land later.
        YOFF0 = DT * W2T
        xy0 = xin.tile([128, 2 * DT * W2T], BF16, tag="xy", name="xy")
        # small constants land first (one cheap descriptor); weights and the
        # first group's input interleave per k-tile so psx(k) can start the
        # moment its own tile + input chunk arrive.
        cst = const.tile([128, 3 * DT + W2T // 2], F32, tag="cst")
        nc.sync.dma_start(out=cst[:], in_=cst_d[:, :])
        b1t = cst[:, 0:DT]
        b2t = cst[:, DT:2 * DT]
        u3 = cst[:, 2 * DT:3 * DT]
        zb = cst[:, 3 * DT:].bitcast(BF16)
        w1t = const.tile([128, DT * C], BF16, tag="w1")
        w2t = const.tile([128, DT * C], BF16, tag="w2")
        w1_sb = [w1t[:, k * C:(k + 1) * C] for k in range(DT)]
        w2_sb = [w2t[:, k * C:(k + 1) * C] for k in range(DT)]
        for k in range(DT):
            nc.sync.dma_start(out=w1_sb[k], in_=w1_d[k * 128:(k + 1) * 128, :])
            if k % 2 == 0:
                c = k // 2
                nc.sync.dma_start(out=xy0[:, c * 2 * W2T:(c + 1) * 2 * W2T],
                                  in_=xy_d[0, :, c * 2 * W2T:(c + 1) * 2 * W2T])
        for k in range(DT):
            nc.sync.dma_start(out=w2_sb[k], in_=w2_d[k * 128:(k + 1) * 128, :])
        nc.sync.dma_start(out=xy0[:, YOFF0:2 * YOFF0],
                          in_=xy_d[0, :, YOFF0:2 * YOFF0])
        # Absorb the bias-tile DMA deps into ACT program order now, so the
        # relu evictions later only ever wait on the PE semaphore (the ISA
        # Activation descriptor holds a single sync-wait).
        ident = const.tile([1, 1], F32, tag="ident")
        nc.vector.memset(ident[:], 1.0)
        warm1 = const.tile([128, 1], F32, tag="warm1")
        warm2 = const.tile([128, 1], F32, tag="warm2")
        nc.scalar.activation(warm1[:], b1t[:, 0:1], ACTF.Copy)
        nc.scalar.activation(warm2[:], b2t[:, 0:1], ACTF.Copy)

        YOFF = DT * W2T

        def emit_prelude(g, xyg=None):
            if xyg is None:
                xyg = xin.tile([128, 2 * DT * W2T], BF16, tag="xy", name="xy")
                nc.sync.dma_start(out=xyg[:, 0:YOFF], in_=xy_d[g, :, 0:YOFF])
                nc.sync.dma_start(out=xyg[:, YOFF:2 * YOFF],
                                  in_=xy_d[g, :, YOFF:2 * YOFF])
            # phixy[i][d]: [128, 392] = [phi_x | phi_y] of sample (2g+i), tile d
            phixy = [[phip.tile([128, W2T], BF16, tag=f"phi_{i}_{d}",
                                name=f"phi_{i}_{d}") for d in range(DT)]
                     for i in range(G)]
            t_sbs = [sp.tile([128, DT], F32, tag=f"t_{i}", name=f"t_{i}")
                     for i in range(G)]
            s_sbs = [sp.tile([128, DT], BF16, tag=f"s_{i}", name=f"s_{i}")
                     for i in range(G)]
            return xyg, phixy, t_sbs, s_sbs

        def emit_mains_d(g, st, d_lo, d_hi):
            xyg, phixy, t_sbs, s_sbs = st
            last = g == NG - 1
            for d in range(d_lo, d_hi):
                psx = ps.tile([128, W2T], F32, tag="psx", name="psx", bufs=2)
                psy = ps.tile([128, W2T], F32, tag="psy", name="psy", bufs=2)
                for k in range(DT):
                    nc.tensor.matmul(
                        psx[:], w1_sb[k][:, d * 128:(d + 1) * 128],
                        xyg[:, k * W2T:(k + 1) * W2T],
                        start=(k == 0), stop=(k == DT - 1))
                for k in range(DT):
                    nc.tensor.matmul(
                        psy[:], w2_sb[k][:, d * 128:(d + 1) * 128],
                        xyg[:, YOFF + k * W2T:YOFF + (k + 1) * W2T],
                        start=(k == 0), stop=(k == DT - 1))
                for i in range(G):
                    nc.scalar.activation(phixy[i][d][:, 0:N], psx[:, i * N:(i + 1) * N],
                                         ACTF.Relu, bias=b1t[:, d:d + 1])
                    nc.scalar.activation(phixy[i][d][:, N:W2T], psy[:, i * N:(i + 1) * N],
                                         ACTF.Relu, bias=b2t[:, d:d + 1])
                # s-reduction rides along per-d: DVE drains while PE moves on
                for i in range(G):
                    scr = sp.tile([128, W2T], BF16, tag=f"scr_{d}", name="scr")
                    nc.vector.scalar_tensor_tensor(
                        out=scr[:], in0=phixy[i][d][:], scalar=1.0, in1=zb[:],
                        op0=ALU.mult, op1=ALU.mult,
                        accum_out=t_sbs[i][:, d:d + 1])
                    if last:
                        nc.vector.tensor_scalar_add(
                            s_sbs[i][:, d:d + 1], t_sbs[i][:, d:d + 1],
                            u3[:, d:d + 1])

        def emit_mains_fin(g, st):
            xyg, phixy, t_sbs, s_sbs = st
            if g != NG - 1:
                for i in range(G):
                    nc.vector.tensor_tensor(s_sbs[i][:], t_sbs[i][:], u3[:],
                                            ALU.add)
            return xyg, s_sbs, phixy

        def emit_tail_a1(g, xyg, s_sbs, phixy):
            """xt prefetch + PE matvec; safe to interleave mid-mains."""
            xts, psws = [], []
            for i in range(G):
                sidx = G * g + i
                xt0 = xtp.tile([N0, 2 * C], BF16, tag=f"xt0_{i}", name=f"xt0_{i}")
                xt1 = xtp.tile([N1, 2 * C], BF16, tag=f"xt1_{i}", name=f"xt1_{i}")
                nc.sync.dma_start(out=xt0[:], in_=xt_d[sidx, 0:N0, :])
                nc.sync.dma_start(out=xt1[:], in_=xt_d[sidx, N0:N, :])
                xts.append((xt0, xt1))
                psw = ps.tile([128, W2T], F32, tag="psw", name="psw", bufs=2)
                for d in range(DT):
                    nc.tensor.matmul(psw[0:1, :], s_sbs[i][:, d:d + 1],
                                     phixy[i][d][:],
                                     start=(d == 0), stop=(d == DT - 1))
                psws.append(psw)
            return xts, psws

        def emit_tail_a2(g, xts, psws):
            """W-row eviction (+c0) and PE transposes into PSUM columns."""
            pst = ps.tile([128, 4 * G], F32, tag="pst", name="pst", bufs=2)
            for i in range(G):
                wrow = sp.tile([1, W2T], F32, tag="wrow", name="wrow")
                nc.scalar.activation(wrow[:], psws[i][0:1, :], ACTF.Copy, bias=c0)
                b = 4 * i
                nc.tensor.transpose(pst[:, b + 0:b + 1], wrow[0:1, 0:N0], ident[:])
                nc.tensor.transpose(pst[:, b + 1:b + 2], wrow[0:1, N:N + N0], ident[:])
                nc.tensor.transpose(pst[0:N1, b + 2:b + 3], wrow[0:1, N0:N], ident[:])
                nc.tensor.transpose(pst[0:N1, b + 3:b + 4], wrow[0:1, N + N0:W2T], ident[:])
            return xts, [pst] * G

        def emit_tail_b(g, xts, wcs):
            """out[tok] = x[tok]*(Wx+c0) + y[tok]*(Wy+c0), token-major."""
            for i in range(G):
                (xt0, xt1), pst = xts[i], wcs[i]
                sidx = G * g + i
                b = 4 * i
                for ci, (xt, p0, pn) in enumerate(((xt0, 0, N0), (xt1, N0, N1))):
                    ot = op.tile([pn, C], BF16, tag=f"ot{ci}_{i}",
                                 name=f"ot{ci}_{i}")
                    tmp = op.tile([pn, C], BF16, tag=f"tm{ci}_{i}",
                                  name=f"tm{ci}_{i}")
                    nc.vector.tensor_scalar_mul(
                        tmp[:], xt[:, 0:C], pst[0:pn, b + 2 * ci:b + 2 * ci + 1])
                    nc.vector.scalar_tensor_tensor(
                        out=ot[:], in0=xt[:, C:2 * C],
                        scalar=pst[0:pn, b + 2 * ci + 1:b + 2 * ci + 2],
                        in1=tmp[:], op0=ALU.mult, op1=ALU.add)
                    nc.sync.dma_start(out=out_d[sidx, p0:p0 + pn, :], in_=ot[:])

        # Three-phase pipeline (lags 1/2). Only the PE matvec of group g-1
        # interleaves into the middle of mains(g) (its s-vector is ready and
        # PSUM double-buffering lets both samples complete); the ACT wrow
        # eviction stays after mains(g) so the RELU stream is never blocked.
        mains, half, tails = {}, {}, {}
        for g in range(NG):
            st = emit_prelude(g, xy0 if g == 0 else None)
            emit_mains_d(g, st, 0, DT // 2)
            if g - 1 in mains:
                half[g - 1] = emit_tail_a1(g - 1, *mains.pop(g - 1))
            emit_mains_d(g, st, DT // 2, DT)
            mains[g] = emit_mains_fin(g, st)
            if g - 1 in half:
                tails[g - 1] = emit_tail_a2(g - 1, *half.pop(g - 1))
            if g - 2 in tails:
                emit_tail_b(g - 2, *tails.pop(g - 2))
        for g in sorted(mains):
            tails[g] = emit_tail_a2(g, *emit_tail_a1(g, *mains.pop(g)))
        for g in sorted(tails):
            emit_tail_b(g, *tails.pop(g))

    _split_multi_waits(nc)
    return nc


def _split_multi_waits(nc):
    """This walrus build accepts at most ONE sync-wait command per TPB
    instruction; the Tile scheduler happily emits several. Hoist all but the
    last wait of each instruction onto same-engine EventSemaphore ops placed
    immediately before it (engine program order is the within-block
    subsequence, so this preserves semantics)."""
    import json
    data = json.loads(nc.to_json_bytes())
    n = 0
    for fn in data["functions"]:
        for blk in fn["blocks"]:
            out = []
            for inst in blk["instructions"]:
                si = inst.get("sync_info")
                ow = (si or {}).get("on_wait") or []
                if len(ow) > 1:
                    for w in ow[:-1]:
                        n += 1
                        out.append({
                            "name": f"eswait_{n}",
                            "opcode": "EventSemaphore",
                            "engine": inst["engine"],
                            "ins": [],
                            "outs": [],
                            "sync_info": {"on_wait": [w], "on_update": []},
                        })
                    si["on_wait"] = [ow[-1]]
                out.append(inst)
            blk["instructions"] = out
    nc.m = mybir.module_from_json_bytes(json.dumps(data).encode())
    return nc


def prep_host(inputs: dict):
    x = np.ascontiguousarray(np.asarray(inputs["x"], dtype=np.float32))
    y = np.ascontiguousarray(np.asarray(inputs["y"], dtype=np.float32))
    W1 = np.asarray(inputs["W1"], dtype=np.float32)
    W2 = np.asarray(inputs["W2"], dtype=np.float32)
    g1 = np.asarray(inputs["g1"], dtype=np.float32)
    g2 = np.asarray(inputs["g2"], dtype=np.float32)
    b1 = np.asarray(inputs["b1"], dtype=np.float32)
    b2 = np.asarray(inputs["b2"], dtype=np.float32)
    be1 = np.asarray(inputs["be1"], dtype=np.float32)
    be2 = np.asarray(inputs["be2"], dtype=np.float32)
    W3 = np.asarray(inputs["W3"], dtype=np.float32)
    b3 = np.asarray(inputs["b3"], dtype=np.float32)
    W4 = np.asarray(inputs["W4"], dtype=np.float32)
    b4 = np.asarray(inputs["b4"], dtype=np.float32)
    W5 = np.asarray(inputs["W5"], dtype=np.float32)
    b5 = np.asarray(inputs["b5"], dtype=np.float32)

    W1p = np.ascontiguousarray(W1 * g1[None, :]).astype(NPBF)
    W2p = np.ascontiguousarray(W2 * g2[None, :]).astype(NPBF)
    b1p = b1 * g1 + be1
    b2p = b2 * g2 + be2
    W5a, W5b = W5[:C, 0], W5[C:, 0]
    u3 = (W3 @ W5a).astype(np.float32)
    u4 = (W4 @ W5b).astype(np.float32)
    z = (u4[:2 * N] + u4[2 * N:]).astype(np.float32)
    c0 = float(b3 @ W5a + b4 @ W5b + b5[0])

    # [B,N,C] -> per-core groups [M, NG, 128, DT*392] with [x_a|x_b] 392-blocks
    def pack(a):
        at = a.transpose(0, 2, 1).reshape(NCORES, S, DT, 128, N)
        pair = at.reshape(NCORES, NG, G, DT, 128, N)
        gg = np.concatenate([pair[:, :, 0], pair[:, :, 1]], axis=-1)  # [M,NG,DT,128,392]
        return np.ascontiguousarray(
            gg.transpose(0, 1, 3, 2, 4).reshape(NCORES, NG, 128, DT * W2T))

    XY = np.concatenate([pack(x), pack(y)], axis=-1).astype(NPBF)
    # token-major second copy for the final reweighting: [M, S, N, 2C]
    XT = np.concatenate([x, y], axis=-1).reshape(NCORES, S, N, 2 * C).astype(NPBF)
    zb = np.broadcast_to(z[None, :], (128, W2T)).astype(NPBF)
    u3t = u3.reshape(DT, 128).T
    b1t = b1p.reshape(DT, 128).T
    b2t = b2p.reshape(DT, 128).T
    cst = np.concatenate(
        [b1t.astype(np.float32), b2t.astype(np.float32),
         u3t.astype(np.float32),
         np.ascontiguousarray(zb).view(np.float32)], axis=1)
    cst = np.ascontiguousarray(cst)

    in_maps = []
    for cidx in range(NCORES):
        in_maps.append({
            "xy": XY[cidx], "xt": XT[cidx], "w1": W1p, "w2": W2p,
            "cst": cst,
        })
    return in_maps, c0, x, y


def unpack_out(results) -> np.ndarray:
    outs = []
    for cidx in range(NCORES):
        o = np.asarray(results[cidx]["out"]).astype(np.float32)  # [S, N, C]
        outs.append(o)
    return np.ascontiguousarray(np.concatenate(outs, axis=0))


def kernel(**inputs) -> np.ndarray:
    in_maps, c0, _, _ = prep_host(inputs)
    nc = build_bass(c0)
    res = run_bass_kernel_spmd(nc, in_maps, list(range(NCORES)))
    return unpack_out(res.results)
